# revision 1
# baseline (speedup 1.0000x reference)
"""Causal multi-head attention on 8 Trainium2 NeuronCores.

Sharding: data-parallel over batch (B=2) x tensor-parallel over heads
(16 heads -> 4 groups of 4). Core (b, hg) computes, for batch b and its
4 heads: Q/K/V projections, causal attention, and a partial output
projection against its slice of Wo. The host sums the 4 partials per
batch (the "all-reduce" of the reference TP recipe, done at unshard).

Per-core dataflow (all matmuls bf16 inputs, fp32 PSUM accumulate):
  QT = Wq_hg @ x_b.T          [256, 2048]   (head dim on partitions)
  KT = Wk_hg @ x_b.T          [256, 2048]
  V  = x_b @ Wv_hg.T          [2048, 256]   (seq on partitions)
  per (head pair, 512-wide q block, 128-wide k tile):
    scoresT[k, q] = K_h @ Q_h.T      (two heads packed in PE rows 0-63/64-127)
    expT = exp(scoresT / 8) * causal_mask          (ACT, bf16 out)
    outT[65, q]  += [V_h | ones].T @ expT          (row 64 = softmax denom)
  normalize via reciprocal + PE broadcast, then
  out_partial = attnT.T @ WoT_hg      [2048, 1024] fp32
"""

import numpy as np
import ml_dtypes

import concourse.bass as bass
import concourse.mybir as mybir
from concourse.tile import TileContext
from concourse.bass_utils import run_bass_kernel_spmd

B, S, D, H = 2, 2048, 1024, 16
NCORES, NHG = 8, 4          # cores, head groups
HL = H // NHG               # 4 heads per core
DK = D // H                 # 64
HD = HL * DK                # 256 local head dims
P = 128
KO = D // P                 # 8 contraction tiles over D
QB = 512                    # q block width
NQB = S // QB               # 4
NKT = S // P                # 16 k tiles
NST = S // P                # 16 seq tiles

bf16 = ml_dtypes.bfloat16
BF, F32, FR = mybir.dt.bfloat16, mybir.dt.float32, mybir.dt.float32r
EXP = mybir.ActivationFunctionType.Exp
MUL = mybir.AluOpType.mult


def _split_multiwaits(nc, max_waits=1):
    # The walrus build in this container accepts at most one sync-wait
    # command per instruction; hoist extra waits onto single-wait NoOps
    # preceding the instruction on the same engine.
    for f in nc.m.functions:
        for bb in f.blocks:
            new = []
            changed = False
            for ins in bb.instructions:
                si = ins.sync_info
                if si is not None and si.on_wait and len(si.on_wait) > max_waits:
                    waits = list(si.on_wait)
                    for k, w in enumerate(waits[:-max_waits]):
                        new.append(mybir.InstNoOp(
                            name=f"{ins.name}-wsplit{k}",
                            engine=ins.engine,
                            sync_info=mybir.SyncInfo(on_wait=[w], on_update=[]),
                            bass_nofuse=True,
                        ))
                    si.on_wait = waits[-max_waits:]
                    changed = True
                new.append(ins)
            if changed:
                bb.instructions = new


def _build():
    nc = bass.Bass()
    xT = nc.dram_tensor("xT", [P, KO, S], BF, kind="ExternalInput")
    wq = nc.dram_tensor("wq", [P, KO, HD], BF, kind="ExternalInput")
    wk = nc.dram_tensor("wk", [P, KO, HD], BF, kind="ExternalInput")
    wv = nc.dram_tensor("wv", [P, KO, HD], BF, kind="ExternalInput")
    wo = nc.dram_tensor("wo", [P, HD // P, D], BF, kind="ExternalInput")
    masks = nc.dram_tensor("masks", [P, 2, P], BF, kind="ExternalInput")
    ones = nc.dram_tensor("ones", [P, DK], FR, kind="ExternalInput")
    out = nc.dram_tensor("out", [S, D], F32, kind="ExternalOutput")

    with TileContext(nc) as tc:
        with (
            tc.tile_pool(name="const", bufs=1) as cp,
            tc.tile_pool(name="work", bufs=5) as wp,
            tc.tile_pool(name="rwork", bufs=4) as rp,
            tc.tile_pool(name="psS", bufs=2, space="PSUM") as psS,
            tc.tile_pool(name="psO", bufs=2, space="PSUM") as psO,
            tc.tile_pool(name="psM", bufs=2, space="PSUM") as psM,
        ):
            # interleave weight/x DMAs per contraction tile so the first
            # projection matmuls can start after ~1/8 of the input traffic
            xT_sb = cp.tile([P, KO, S], BF, tag="xT")
            wq_sb = cp.tile([P, KO, HD], BF, tag="wq")
            wk_sb = cp.tile([P, KO, HD], BF, tag="wk")
            wv_sb = cp.tile([P, KO, HD], BF, tag="wv")
            nc.sync.dma_start(wq_sb[:], wq[:])
            # x column-block 0 right after Wq so the first QT tiles can
            # start early; Wk follows before the Q tiles run out
            for k in range(KO):
                nc.sync.dma_start(xT_sb[:, k, bass.ts(0, QB)],
                                  xT[:, k, bass.ts(0, QB)])
            nc.sync.dma_start(wk_sb[:], wk[:])
            for n in range(1, S // QB):
                for k in range(KO):
                    nc.sync.dma_start(xT_sb[:, k, bass.ts(n, QB)],
                                      xT[:, k, bass.ts(n, QB)])
            nc.sync.dma_start(wv_sb[:], wv[:])
            wo_sb = cp.tile([P, HD // P, D], BF, tag="wo")
            nc.sync.dma_start(wo_sb[:], wo[:])
            mk_sb = cp.tile([P, 2, P], BF, tag="mk")
            nc.sync.dma_start(mk_sb[:], masks[:])

            ones_sb = cp.tile([P, DK], FR, tag="ones")
            nc.sync.dma_start(ones_sb[:], ones[:])

            QT_sb = cp.tile([P, HD // P, S], BF, tag="QT")
            KT_sb = cp.tile([P, HD // P, S], BF, tag="KT")
            # V with a ones column appended per head: [p, seq_tile, head, 65]
            va_sb = cp.tile([P, NST, HL, DK + 1], BF, tag="va")
            nc.vector.memset(va_sb[:, :, :, DK:DK + 1], 1.0)
            attnT_sb = cp.tile([P, HD // P, S], BF, tag="attnT")

            # prime the ACT exp table set while PE runs the projections
            warm = rp.tile([1, 8], F32, tag="warm")
            nc.vector.memset(warm[:], 0.0)
            nc.scalar.activation(warm[:], warm[:], EXP)

            # ---- projections ----
            for n in range(S // QB):
                ns = bass.ts(n, QB)
                for m in range(HD // P):
                    pq = psM.tile([P, QB], F32, tag="ps1")
                    for k in range(KO):
                        nc.tensor.matmul(pq[:], wq_sb[:, k, bass.ts(m, P)],
                                         xT_sb[:, k, ns],
                                         start=(k == 0), stop=(k == KO - 1))
                    nc.vector.tensor_copy(QT_sb[:, m, ns], pq[:])
                for m in range(HD // P):
                    pk = psM.tile([P, QB], F32, tag="ps1")
                    for k in range(KO):
                        nc.tensor.matmul(pk[:], wk_sb[:, k, bass.ts(m, P)],
                                         xT_sb[:, k, ns],
                                         start=(k == 0), stop=(k == KO - 1))
                    nc.scalar.copy(KT_sb[:, m, ns], pk[:])
            for st in range(NST):
                pv = psM.tile([P, QB], F32, tag="ps1")
                for k in range(KO):
                    nc.tensor.matmul(pv[:, :HD], xT_sb[:, k, bass.ts(st, P)],
                                     wv_sb[:, k], start=(k == 0), stop=(k == KO - 1))
                nc.vector.tensor_copy(
                    va_sb[:, st, :, 0:DK],
                    pv[:, :HD].rearrange("p (h d) -> p h d", d=DK))

            # ---- attention + output projection, per q block ----
            # Normalization and Wo matmuls are deferred into the NEXT
            # group's score-matmul stream: the PE executes in order, so a
            # matmul whose input (reciprocal on DVE / attnT mult) isn't
            # ready yet would head-of-line-block the queue.
            norm_jobs = []   # flushed right after the next group's first scores
            wo_jobs = []     # dribbled one per k-tile iteration

            def norm_job(po, hp, hh, qs, tail=False):
                # copy the accumulator to SBUF (frees its PSUM bank early and
                # lets the final mult read PSUM only once — walrus forbids two
                # PSUM operands), take 1/sums, broadcast across partitions
                # with a rank-1 fp32r matmul, multiply. The tail flavor skips
                # the accumulator copy (shorter critical path) and routes the
                # broadcast copy through the by-then-idle ScalarEngine.
                def run():
                    rc = rp.tile([DK + 1, QB], FR, tag="rc", name="rc")
                    if tail:
                        src_att = po[0:DK]
                        with nc.allow_low_precision(reason="fp32r is fp32-width"):
                            nc.vector.reciprocal(rc[DK:DK + 1], po[DK:DK + 1])
                    else:
                        poc = rp.tile([DK + 1, QB], F32, tag="poc", name="poc")
                        nc.vector.tensor_copy(poc[:], po[:])
                        src_att = poc[0:DK]
                        with nc.allow_low_precision(reason="fp32r is fp32-width"):
                            nc.vector.reciprocal(rc[DK:DK + 1], poc[DK:DK + 1])
                    pb = psM.tile([P, QB], F32, tag="ps1", name="pb")
                    nc.tensor.matmul(pb[:DK], ones_sb[DK:DK + 1],
                                     rc[DK:DK + 1], start=True, stop=True)
                    if tail:
                        bc = rp.tile([DK, QB], F32, tag="bc", name="bc")
                        nc.scalar.copy(bc[:], pb[:DK])
                        nc.vector.tensor_tensor(
                            attnT_sb[hh * DK:(hh + 1) * DK, hp, qs],
                            po[0:DK], bc[:], MUL)
                    else:
                        nc.vector.tensor_tensor(
                            attnT_sb[hh * DK:(hh + 1) * DK, hp, qs],
                            src_att, pb[:DK], MUL)
                return run

            def wo_job(st, n):
                def run():
                    pw = psM.tile([P, QB], F32, tag="ps1", name="pw")
                    for i in range(HD // P):
                        nc.tensor.matmul(pw[:], attnT_sb[:, i, bass.ts(st, P)],
                                         wo_sb[:, i, bass.ts(n, QB)],
                                         start=(i == 0), stop=(i == HD // P - 1))
                    ot = wp.tile([P, QB], F32, tag="out", name="ot")
                    nc.vector.tensor_copy(ot[:], pw[:])
                    nc.sync.dma_start(out[bass.ts(st, P), bass.ts(n, QB)], ot[:])
                return run

            # PV matmuls are emitted one k-tile iteration late (and carried
            # across group boundaries) so the in-order PE never waits on the
            # exp (ACT) that feeds them.
            pending_pv = []

            def pv_job(po, kt, ex, off, nkt, hp):
                def run():
                    for hh in range(2):
                        nc.tensor.matmul(po[hh][:, off:],
                                         va_sb[:, kt, 2 * hp + hh],
                                         ex[:, hh, off:],
                                         start=(kt == 0), stop=(kt == nkt - 1))
                return run

            for qb in range(NQB):
                nkt = 4 * (qb + 1)
                qs = bass.ts(qb, QB)
                for hp in range(HD // P):        # head pair (2 heads per 128 partitions)
                    po = [psO.tile([DK + 1, QB], F32, tag="psO", name=f"po{qb}_{hp}_{i}")
                          for i in range(2)]
                    for kt in range(nkt):
                        # columns q < kt*128 of this q block are fully causal-
                        # masked: skip them in scores/exp/PV entirely
                        off = max(0, (kt - 4 * qb) * P)
                        w = QB - off
                        ps = psS.tile([P, 2, QB], F32, tag="psS")
                        ex = wp.tile([P, 2, QB], BF, tag="exp")
                        for hh in range(2):
                            hsl = slice(hh * DK, (hh + 1) * DK)
                            nc.tensor.matmul(ps[:, hh, off:],
                                             KT_sb[hsl, hp, bass.ts(kt, P)],
                                             QT_sb[hsl, hp, bass.ds(qb * QB + off, w)],
                                             start=True, stop=True)
                        while pending_pv:
                            pending_pv.pop(0)()
                        if kt <= 1 and norm_jobs:
                            norm_jobs.pop(0)()
                        nc.scalar.activation(ex[:, :, off:], ps[:, :, off:],
                                             EXP, scale=1.0 / 8.0)
                        if kt >= 4 * qb:
                            # only the leading 128 remaining columns straddle
                            # the diagonal; later ones are fully visible
                            nc.vector.tensor_tensor(ex[:, :, off:off + P],
                                                    ex[:, :, off:off + P],
                                                    mk_sb[:], MUL)
                        pending_pv.append(pv_job(po, kt, ex, off, nkt, hp))
                        if kt >= 1 and kt % 2 == 1 and wo_jobs:
                            wo_jobs.pop(0)()
                    for hh in range(2):
                        norm_jobs.append(norm_job(
                            po[hh], hp, hh, qs,
                            tail=(qb == NQB - 1 and hp == HD // P - 1)))
                wo_jobs.extend(wo_job(st, n)
                               for st in range(4 * qb, 4 * qb + 4)
                               for n in range(D // QB))
            while pending_pv:
                pending_pv.pop(0)()
            for j in norm_jobs:
                j()
            for j in wo_jobs:
                j()

    _split_multiwaits(nc)
    return nc


_NC_CACHE = []


def _prepare_in_maps(x, Wq, Wk, Wv, Wo):
    def tile_k(a, free):
        # [D, free] -> [P, KO_like, free] partition-tiled bf16
        ko = a.shape[0] // P
        return np.ascontiguousarray(
            a.reshape(ko, P, free).transpose(1, 0, 2)).astype(bf16)

    # causal triangle for the diagonal 128-col strip, duplicated for the
    # two packed heads
    tri = (np.arange(P)[:, None] <= np.arange(P)[None, :]).astype(np.float32)
    mk = np.stack([tri, tri], axis=1).astype(bf16)

    in_maps = []
    for core in range(NCORES):
        b, hg = divmod(core, NHG)
        sl = slice(hg * HD, (hg + 1) * HD)
        xb = np.asarray(x[b], np.float32)
        in_maps.append({
            "xT": tile_k(xb.T, S),
            "wq": tile_k(np.asarray(Wq[sl], np.float32).T, HD),
            "wk": tile_k(np.asarray(Wk[sl], np.float32).T, HD),
            "wv": tile_k(np.asarray(Wv[sl], np.float32).T, HD),
            "wo": tile_k(np.asarray(Wo[:, sl], np.float32).T, D),
            "masks": mk,
            "ones": np.ones((P, DK), np.float32),
        })
    return in_maps


def kernel(x, Wq, Wk, Wv, Wo):
    if not _NC_CACHE:
        _NC_CACHE.append(_build())
    nc = _NC_CACHE[0]
    in_maps = _prepare_in_maps(x, Wq, Wk, Wv, Wo)
    res = run_bass_kernel_spmd(nc, in_maps, core_ids=list(range(NCORES)))
    out = np.zeros((B, S, D), np.float32)
    for core in range(NCORES):
        out[core // NHG] += res.results[core]["out"]
    return out


def hw_time(inputs, iters=24):
    """Test-only helper: measure per-execution device time by issuing the
    compiled NEFF back-to-back with resident device inputs (no donation, so
    buffers are reusable) and fitting the per-iteration slope. The axon NTFF
    profiling hook isn't available in this container, so this amortized
    wall-clock slope is the closest proxy for HW exec time."""
    import time
    import jax
    from concourse import bass2jax
    import concourse.mybir as mybir_

    if not _NC_CACHE:
        _NC_CACHE.append(_build())
    nc = _NC_CACHE[0]
    in_maps = _prepare_in_maps(**inputs)

    bass2jax.install_neuronx_cc_hook()
    pid_name = nc.partition_id_tensor.name if nc.partition_id_tensor else None
    in_names, out_names, out_avals, zero_outs = [], [], [], []
    for alloc in nc.m.functions[0].allocations:
        if not isinstance(alloc, mybir_.MemoryLocationSet):
            continue
        name = alloc.memorylocations[0].name
        if alloc.kind == "ExternalInput":
            if name != pid_name:
                in_names.append(name)
        elif alloc.kind == "ExternalOutput":
            out_names.append(name)
            shape = tuple(alloc.tensor_shape)
            dtype = mybir_.dt.np(alloc.dtype)
            out_avals.append(jax.core.ShapedArray(shape, dtype))
            zero_outs.append(np.zeros(shape, dtype))
    n_params = len(in_names)
    all_names = in_names + out_names
    if pid_name is not None:
        all_names = all_names + [pid_name]

    def _body(*args):
        operands = list(args)
        if pid_name is not None:
            operands.append(bass2jax.partition_id_tensor())
        outs = bass2jax._bass_exec_p.bind(
            *operands,
            out_avals=tuple(out_avals),
            in_names=tuple(all_names),
            out_names=tuple(out_names),
            lowering_input_output_aliases=(),
            sim_require_finite=True,
            sim_require_nnan=True,
            nc=nc,
        )
        return tuple(outs)

    devices = jax.devices()[:NCORES]
    mesh = bass2jax.Mesh(np.asarray(devices), ("core",))
    spec = bass2jax.PartitionSpec("core")
    n_args = n_params + len(out_names)
    fn = jax.jit(bass2jax.shard_map(
        _body, mesh=mesh, in_specs=(spec,) * n_args,
        out_specs=(spec,) * len(out_names), check_rep=False))
    sharding = jax.sharding.NamedSharding(mesh, spec)
    concat_in = [
        jax.device_put(
            np.concatenate([np.asarray(in_maps[c][nm]) for c in range(NCORES)], axis=0),
            sharding)
        for nm in in_names
    ]
    concat_zeros = [
        jax.device_put(np.zeros((NCORES * z.shape[0], *z.shape[1:]), z.dtype), sharding)
        for z in zero_outs
    ]
    # warm up (compile + first exec)
    jax.block_until_ready(fn(*concat_in, *concat_zeros))

    def run_n(n):
        t0 = time.perf_counter()
        o = None
        for _ in range(n):
            o = fn(*concat_in, *concat_zeros)
        jax.block_until_ready(o)
        return time.perf_counter() - t0

    slopes = []
    for _ in range(5):
        t1 = run_n(16)
        t2 = run_n(64)
        slopes.append((t2 - t1) / 48)
    # min slope = least host/tunnel contention; still includes per-launch
    # runtime overhead, so it upper-bounds the true kernel span
    return int(min(slopes) * 1e9)

